# revision 33
# baseline (speedup 1.0000x reference)
"""MinGRU block (RMSNorm -> minGRU scan -> residual -> RMSNorm -> SwiGLU FFN
-> residual) for Trainium2, SPMD over 8 NeuronCores.

Sharding: core c handles batch b=c//2, token-half s=c%2 (T_my=2048 tokens
each). The scan needs its carry-in, but the gate g=sigmoid(z+1) contracts
state by ~e^-0.35 per step, so a W=128-token warmup run with h=0 initial
reconstructs the carry to ~1e-19 — instead of recomputing the whole first
half, each core's phase-1 program covers only W + T_my = 2176 tokens. s=0
cores get W zeros in front (zero rows keep the scan state exactly 0 since
bc==0); s=1 cores get the real tokens [T_my-W, T_my).

Precision (chosen from a numpy error model of the full chain, tol 2e-2):
gates/cands and the W2 down-proj run in bf16; only the W1/W3 up-projs run
as fp8-e4m3 DoubleRow matmuls (2x PE throughput) — fp8 noise in the scan
path and the last FFN stage is what blows the error budget, while the
silu/mult stage tolerates it (model: 1.4e-2 vs 2.6e-2 all-fp8). W1/W3 are
host-quantized with power-of-2 scales folded into the silu scale and the
fused ffp rescale; fin is quantized on the fly by the norm-apply multiply
writing fp8.

Everything on-device is feature-major [D, tokens]; matmul outputs land as
[out_channel, tokens] — the layout the per-channel scan wants. RMSNorm's
partition-dim reduce/broadcast go through the tensor engine as bf16
ones-matmuls (1 cycle/row vs f32's 4). Each phase-1 chunk's norm is split:
the squares (ACT) are emitted BEFORE the current chunk's matmul body and
the reduce/apply (PE/DVE) AFTER it, so the in-order PE queue never blocks
on a square that hasn't run. x+h is kept only as bf16 in SBUF (no f32
DRAM spill): it feeds both the phase-2 norm and the final residual.
Residual adds run on GpSimd writing bf16 directly; gates/cands/scan in
bf16 on DVE; silu on the ACT LUT. Phase-2 weight tiles are loaded one
step ahead, and block 1's norm is prepared between block 0's two matmul
stages.
"""

import os
import sys

sys.path.insert(0, "/opt/trn_rl_repo")

from contextlib import ExitStack

import ml_dtypes
import numpy as np

import concourse.bass as bass
import concourse.mybir as mybir
from concourse import bacc
from concourse.tile import TileContext

P = 128
EPS = 1e-6
F32 = mybir.dt.float32
BF16 = mybir.dt.bfloat16
FP8 = mybir.dt.float8e4
E4NP = ml_dtypes.float8_e4m3
MULT = mybir.AluOpType.mult
ADD = mybir.AluOpType.add
SUB = mybir.AluOpType.subtract
AF = mybir.ActivationFunctionType
DR = mybir.MatmulPerfMode.DoubleRow


def build_nc(D, DFF, T_my, W, s1, s3, s2, sp, CH=512, BLK=1024):
    """Build the per-core program. s1/s3/s2 are the fp8 W1/W3/W2-half
    scales (powers of two) baked into the silu scale and the fused
    rescales; sp is the extra scale on the quantized silu*z3 product."""
    kd = D // P            # K-chunks over D
    mf = DFF // P          # m-tiles over DFF
    chunks = [(0, W)] + [(W + i * CH, CH) for i in range(T_my // CH)]
    n_ch = len(chunks)
    n_blk = T_my // BLK
    NS = min(512, BLK)     # matmul/psum free-dim sub-chunk
    nspl = BLK // NS
    Lp = W + T_my

    nc = bacc.Bacc("TRN2")
    # x is shipped bf16: halves the dominant input DMA; both consumers
    # (norm input, x+h residual) already round to bf16 anyway
    xt = nc.dram_tensor("xt", (P, kd, Lp), BF16, kind="ExternalInput")
    wg = nc.dram_tensor("wg", (P, kd, D), BF16, kind="ExternalInput")
    wc = nc.dram_tensor("wc", (P, kd, D), BF16, kind="ExternalInput")
    bias = nc.dram_tensor("bias", (P, 2, kd), F32, kind="ExternalInput")
    w1 = nc.dram_tensor("w1", (P, kd, DFF), FP8, kind="ExternalInput")
    w3 = nc.dram_tensor("w3", (P, kd, DFF), FP8, kind="ExternalInput")
    # W2 split: first half of the DFF contraction fp8 (DoubleRow), second
    # half bf16 pre-scaled by s2*sp on the host so both halves accumulate
    # into one PSUM group with a single final rescale
    w2a = nc.dram_tensor("w2a", (P, mf // 2, D), FP8, kind="ExternalInput")
    w2b = nc.dram_tensor("w2b", (P, mf // 2, D), BF16, kind="ExternalInput")
    y = nc.dram_tensor("y", (P, kd, T_my), F32, kind="ExternalOutput")

    with TileContext(nc) as tc, ExitStack() as ctx:
        consts = ctx.enter_context(tc.tile_pool(name="consts", bufs=1))
        ones_k = consts.tile([P, 1], BF16)
        nc.vector.memset(ones_k[:], 1.0)
        ones_b = consts.tile([1, P], BF16)
        nc.vector.memset(ones_b[:], 1.0)
        eps_t = consts.tile([1, 1], F32)
        nc.vector.memset(eps_t[:], EPS)
        bias_s = consts.tile([P, 2, kd], F32)
        nc.sync.dma_start(bias_s[:], bias[:])
        # warm the ACT LUT tables during the (DMA-bound, ACT-idle)
        # preamble so the first real uses in the ramp don't pay a 1.3us
        # table load in the critical norm/activation chain. Tanh first:
        # with ~3 table slots it's the one evicted, and its first real
        # use is latest.
        warm = consts.tile([1, 1], F32)
        for wf in (AF.Tanh, AF.Sigmoid, AF.Abs_reciprocal_sqrt):
            nc.scalar.activation(warm[:], eps_t[:], wf)
        nc.scalar.square(warm[:], eps_t[:])

        # bf16 x+h kept in SBUF: feeds the phase-2 norm AND the final
        # residual (no f32 DRAM spill).
        handoff = ctx.enter_context(tc.tile_pool(name="handoff", bufs=1))
        xnew_bf = handoff.tile([P, kd, T_my], BF16)
        rinv_my = handoff.tile([1, T_my], BF16)
        fin0 = handoff.tile([P, kd, BLK], FP8)
        # phase-2 first up-proj weight tiles (4 mts), prefetched during
        # phase 1: their matmuls run at the end of phase 1 so the PE rolls
        # through the phase boundary without a pool-handoff stall
        pre_w1 = handoff.tile([P, kd, 4 * P], FP8)
        pre_w3 = handoff.tile([P, kd, 4 * P], FP8)
        ffq_pre = handoff.tile([P, 4, BLK], FP8)

        def norm_reduce(src, rinv, sqpool, npsum, width):
            # 1/rms of src [P, kd, width] over the channel axis -> rinv
            # [1, width] bf16. Squares on ScalarE (bf16 out); the
            # partition reduce is a ones-matmul (bf16: 1 cycle/row).
            for o in range(0, width, 512):
                w_ = min(512, width - o)
                sl = slice(o, o + w_)
                ssq = npsum.tile([1, 512], F32, name="ssq")[:, :w_]
                for k in range(kd):
                    sq = sqpool.tile([P, 512], BF16, name="sq")[:, :w_]
                    nc.scalar.square(sq, src[:, k, sl])
                    nc.tensor.matmul(ssq, ones_k[:], sq,
                                     start=(k == 0), stop=(k == kd - 1))
                # HW-measured max rel err 4e-5 for this LUT
                nc.scalar.activation(rinv[:, sl], ssq,
                                     AF.Abs_reciprocal_sqrt,
                                     bias=eps_t[:], scale=1.0 / D)

        def norm_apply(src, rinv, out, bpsum, width):
            # out = src * broadcast(rinv) (K=1 ones-matmul broadcast);
            # out dtype from the tile (fp8 -> on-the-fly quantization)
            for o in range(0, width, 512):
                w_ = min(512, width - o)
                sl = slice(o, o + w_)
                rb = bpsum.tile([P, 512], F32, name="rb")[:, :w_]
                nc.tensor.matmul(rb, ones_b[:], rinv[:, sl],
                                 start=True, stop=True)
                for k in range(kd):
                    nc.vector.tensor_mul(out[:, k, sl], src[:, k, sl], rb)

        # ---------------- phase 1: gates/cands + scan ----------------
        with (
            tc.tile_pool(name="p1w", bufs=1) as wpool,
            tc.tile_pool(name="p1x", bufs=4) as xpool,
            tc.tile_pool(name="p1hin", bufs=3) as hinpool,
            tc.tile_pool(name="p1sq", bufs=10) as sqpool,
            tc.tile_pool(name="p1s", bufs=2) as spool,
            tc.tile_pool(name="p1scr", bufs=4) as scr,
            tc.tile_pool(name="p1h", bufs=2) as hpool,
            tc.tile_pool(name="p1np", bufs=2, space="PSUM") as npsum,
            tc.tile_pool(name="p1bp", bufs=2, space="PSUM") as bpsum,
            tc.tile_pool(name="p1zp", bufs=2, space="PSUM") as zpsum,
        ):
            xts, sqmap, hins = {}, {}, {}

            def load(ci):
                off, sz = chunks[ci]
                xt_c = xpool.tile([P, kd, sz], BF16, name="xt_c")
                for k in range(kd):
                    nc.sync.dma_start(xt_c[:, k, :], xt[:, k, off:off + sz])
                xts[ci] = xt_c

            def chunk_squares(ci):
                # ACT part of the chunk norm, emitted ahead of the current
                # chunk's matmul body
                _, sz = chunks[ci]
                sqs = []
                for k in range(kd):
                    sq = sqpool.tile([P, sz], BF16, name="sq")
                    nc.scalar.square(sq, xts[ci][:, k, :])
                    sqs.append(sq)
                sqmap[ci] = sqs

            def chunk_finish(ci):
                # PE reduce + broadcast + DVE apply, emitted after the
                # current chunk's matmul body (PE reaches it with the
                # squares long since done)
                _, sz = chunks[ci]
                sqs = sqmap.pop(ci)
                rinv = spool.tile([1, sz], BF16, name="rinv")
                ssq = npsum.tile([1, CH], F32, name="ssq")[:, :sz]
                for k in range(kd):
                    nc.tensor.matmul(ssq, ones_k[:], sqs[k],
                                     start=(k == 0), stop=(k == kd - 1))
                nc.scalar.activation(rinv[:], ssq, AF.Abs_reciprocal_sqrt,
                                     bias=eps_t[:], scale=1.0 / D)
                hin = hinpool.tile([P, kd, sz], BF16, name="hin")
                rb = bpsum.tile([P, CH], F32, name="rb")[:, :sz]
                nc.tensor.matmul(rb, ones_b[:], rinv[:],
                                 start=True, stop=True)
                for k in range(kd):
                    nc.vector.tensor_mul(hin[:, k, :], xts[ci][:, k, :], rb)
                hins[ci] = hin

            load(0)
            load(1)
            # wg/wc loaded in per-m-column slices: only the m=0 slice
            # gates the first matmuls, and chunk 2's data (needed by its
            # squares at ~the same time) isn't stuck behind 4MB of weights
            wg_s = wpool.tile([P, kd, D], BF16)
            wc_s = wpool.tile([P, kd, D], BF16)
            nc.sync.dma_start(wg_s[:, :, 0:P], wg[:, :, 0:P])
            nc.sync.dma_start(wc_s[:, :, 0:P], wc[:, :, 0:P])
            load(2)
            for m_ in range(1, kd):
                ms_ = slice(m_ * P, (m_ + 1) * P)
                nc.sync.dma_start(wg_s[:, :, ms_], wg[:, :, ms_])
                nc.sync.dma_start(wc_s[:, :, ms_], wc[:, :, ms_])
            # phase-2 prefetches ride behind the phase-1 weights
            nc.sync.dma_start(pre_w1[:], w1[:, :, 0:4 * P])
            nc.sync.dma_start(pre_w3[:], w3[:, :, 0:4 * P])
            chunk_squares(0)
            chunk_finish(0)
            chunk_squares(1)
            chunk_finish(1)

            h_prev = None
            prev_sz = None
            for ci in range(n_ch):
                off, sz = chunks[ci]
                if ci + 3 < n_ch:
                    load(ci + 3)
                if ci + 2 < n_ch:
                    chunk_squares(ci + 2)
                xt_c, hin = xts.pop(ci), hins.pop(ci)

                h_t = hpool.tile([P, kd, sz], BF16, name="h_t")
                for m in range(kd):
                    ms = slice(m * P, (m + 1) * P)
                    zg = zpsum.tile([P, CH], F32, name="zg")[:, :sz]
                    zc = zpsum.tile([P, CH], F32, name="zc")[:, :sz]
                    for k in range(kd):
                        nc.tensor.matmul(zg, wg_s[:, k, ms], hin[:, k, :],
                                         start=(k == 0), stop=(k == kd - 1))
                    for k in range(kd):
                        nc.tensor.matmul(zc, wc_s[:, k, ms], hin[:, k, :],
                                         start=(k == 0), stop=(k == kd - 1))
                    g_t = scr.tile([P, sz], BF16, name="g_t")
                    nc.scalar.activation(g_t, zg, AF.Sigmoid,
                                         bias=bias_s[:, 0, m:m + 1])
                    c_t = scr.tile([P, sz], BF16, name="c_t")
                    nc.scalar.activation(c_t, zc, AF.Tanh,
                                         bias=bias_s[:, 1, m:m + 1])
                    # bn = (g-1)*c = -(1-g)*c in ONE vector op; the scan
                    # uses op1=subtract so state = g*state - bn = g*state
                    # + (1-g)*c
                    b_t = scr.tile([P, sz], BF16, name="b_t")
                    nc.vector.scalar_tensor_tensor(
                        b_t, g_t, 1.0, c_t, op0=SUB, op1=MULT)
                    init = (0.0 if h_prev is None
                            else h_prev[:, m, prev_sz - 1:prev_sz])
                    nc.vector.tensor_tensor_scan(
                        h_t[:, m, :], g_t, b_t, init, op0=MULT, op1=SUB)
                    # emit the next-next chunk's norm reduce mid-body: its
                    # rsqrt drains behind this chunk's remaining
                    # sigmoid/tanh while the PE still has matmuls queued,
                    # so the rinv broadcast never stalls the PE
                    if m == kd // 2 - 1 and ci + 2 < n_ch:
                        chunk_finish(ci + 2)
                h_prev = h_t
                prev_sz = sz

                if ci >= 1:
                    o = off - W
                    for k in range(kd):
                        # residual x+h on the (otherwise idle) GpSimd
                        # engine, writing bf16 directly
                        nc.gpsimd.tensor_add(xnew_bf[:, k, o:o + sz],
                                             xt_c[:, k, :], h_t[:, k, :])
                # once a phase-2 block's tokens have been complete for a
                # full chunk, compute its norm scale (and fin for block 0)
                # ahead of phase 2
                if ci >= 2 and (o := off - W) % BLK == 0 and o >= BLK:
                    b0 = o - BLK
                    norm_reduce(xnew_bf[:, :, b0:b0 + BLK],
                                rinv_my[:, b0:b0 + BLK],
                                sqpool, npsum, BLK)
                    if b0 == 0:
                        norm_apply(xnew_bf[:, :, b0:b0 + BLK],
                                   rinv_my[:, b0:b0 + BLK], fin0,
                                   bpsum, BLK)

            # blk0's first 4 up-proj mts, emitted while the phase-1 pools
            # are still open: they need only fin0 + prefetched weights,
            # and their PSUM gens rotate inside the zg/zc rings, so the PE
            # crosses the phase boundary with no pool-close barrier
            for mt in range(4):
                mts = slice(mt * P, (mt + 1) * P)
                for h in range(BLK // NS):
                    hs = slice(h * NS, (h + 1) * NS)
                    zf1 = zpsum.tile([P, CH], F32, name="zg")[:, :NS]
                    zf3 = zpsum.tile([P, CH], F32, name="zc")[:, :NS]
                    for k2 in range(kd // 2):
                        ks = slice(2 * k2, 2 * k2 + 2)
                        nc.tensor.matmul(zf1, pre_w1[:, ks, mts],
                                         fin0[:, ks, hs],
                                         start=(k2 == 0),
                                         stop=(k2 == kd // 2 - 1),
                                         perf_mode=DR)
                    for k2 in range(kd // 2):
                        ks = slice(2 * k2, 2 * k2 + 2)
                        nc.tensor.matmul(zf3, pre_w3[:, ks, mts],
                                         fin0[:, ks, hs],
                                         start=(k2 == 0),
                                         stop=(k2 == kd // 2 - 1),
                                         perf_mode=DR)
                    sf = scr.tile([P, NS], BF16, name="sf")
                    nc.scalar.activation(sf, zf1, AF.Silu, scale=1.0 / s1)
                    nc.vector.scalar_tensor_tensor(
                        ffq_pre[:, mt, hs], zf3, sp / s3, sf,
                        op0=MULT, op1=MULT)

        # ---------------- phase 2: SwiGLU FFN ----------------
        with (
            tc.tile_pool(name="p2fin", bufs=1) as finpool,
            tc.tile_pool(name="p2w", bufs=3) as wstr,
            tc.tile_pool(name="p2w2", bufs=2) as w2str,
            tc.tile_pool(name="p2ffp", bufs=1) as ffppool,
            tc.tile_pool(name="p2sf", bufs=3) as sfscr,
            tc.tile_pool(name="p2y", bufs=3) as ypool,
            tc.tile_pool(name="p2bp", bufs=1, space="PSUM") as bpsum2,
            tc.tile_pool(name="p2fp", bufs=2, space="PSUM") as fpsum,
            tc.tile_pool(name="p2op", bufs=2, space="PSUM") as opsum,
        ):
            fin_next = None
            w13 = {}

            def wload(mt):
                mts = slice(mt * P, (mt + 1) * P)
                w1_t = wstr.tile([P, kd, P], FP8, name="w1_t")
                nc.sync.dma_start(w1_t[:], w1[:, :, mts])
                w3_t = wstr.tile([P, kd, P], FP8, name="w3_t")
                nc.sync.dma_start(w3_t[:], w3[:, :, mts])
                w13[mt] = (w1_t, w3_t)

            for blk in range(n_blk):
                fin = fin0 if blk == 0 else fin_next
                # first W2 tiles DMA'd now: needed only after the whole
                # up-proj stage, so the load is fully hidden
                w2a_first = w2str.tile([P, mf // 2, P], FP8, name="w2a_t")
                nc.sync.dma_start(w2a_first[:], w2a[:, :, 0:P])
                w2b_first = w2str.tile([P, mf // 2, P], BF16, name="w2b_t")
                nc.sync.dma_start(w2b_first[:], w2b[:, :, 0:P])

                ffq = ffppool.tile([P, mf // 2, BLK], FP8)
                ffb = ffppool.tile([P, mf // 2, BLK], BF16)
                mt0 = 4 if blk == 0 else 0
                wload(mt0)
                for mt in range(mt0, mf):
                    if mt + 1 < mf:
                        wload(mt + 1)
                    w1_t, w3_t = w13.pop(mt)
                    for h in range(nspl):
                        hs = slice(h * NS, (h + 1) * NS)
                        zf1 = fpsum.tile([P, NS], F32, name="zf1")
                        zf3 = fpsum.tile([P, NS], F32, name="zf3")
                        for k2 in range(kd // 2):
                            ks = slice(2 * k2, 2 * k2 + 2)
                            nc.tensor.matmul(zf1, w1_t[:, ks, :],
                                             fin[:, ks, hs],
                                             start=(k2 == 0),
                                             stop=(k2 == kd // 2 - 1),
                                             perf_mode=DR)
                        for k2 in range(kd // 2):
                            ks = slice(2 * k2, 2 * k2 + 2)
                            nc.tensor.matmul(zf3, w3_t[:, ks, :],
                                             fin[:, ks, hs],
                                             start=(k2 == 0),
                                             stop=(k2 == kd // 2 - 1),
                                             perf_mode=DR)
                        # sf = silu(z1) via the ACT LUT (frees the DVE);
                        # ffp = (z3/s3)*sf in one fused op — fp8 (scaled
                        # by sp) for the DoubleRow half of W2, bf16 for
                        # the precision-critical half
                        sf = sfscr.tile([P, NS], BF16, name="sf")
                        nc.scalar.activation(sf, zf1, AF.Silu,
                                             scale=1.0 / s1)
                        if mt < mf // 2:
                            nc.vector.scalar_tensor_tensor(
                                ffq[:, mt, hs], zf3, sp / s3, sf,
                                op0=MULT, op1=MULT)
                        else:
                            nc.vector.scalar_tensor_tensor(
                                ffb[:, mt - mf // 2, hs], zf3, 1.0 / s3,
                                sf, op0=MULT, op1=MULT)

                # prepare the NEXT block's fin here: its squares/reduce
                # slot in while the PE drains this block's up-projs, so
                # the next block starts with fin ready
                if blk + 1 < n_blk:
                    nxt = slice((blk + 1) * BLK, (blk + 2) * BLK)
                    fin_next = finpool.tile([P, kd, BLK], FP8)
                    norm_reduce(xnew_bf[:, :, nxt], rinv_my[:, nxt],
                                sfscr, bpsum2, BLK)
                    norm_apply(xnew_bf[:, :, nxt], rinv_my[:, nxt],
                               fin_next, bpsum2, BLK)

                for m in range(kd):
                    ms = slice(m * P, (m + 1) * P)
                    if m == 0:
                        w2a_t, w2b_t = w2a_first, w2b_first
                    else:
                        w2a_t = w2str.tile([P, mf // 2, P], FP8,
                                           name="w2a_t")
                        nc.sync.dma_start(w2a_t[:], w2a[:, :, ms])
                        w2b_t = w2str.tile([P, mf // 2, P], BF16,
                                           name="w2b_t")
                        nc.sync.dma_start(w2b_t[:], w2b[:, :, ms])
                    for h in range(nspl):
                        hs = slice(h * NS, (h + 1) * NS)
                        gs = slice(blk * BLK + h * NS,
                                   blk * BLK + (h + 1) * NS)
                        zo = opsum.tile([P, NS], F32)
                        for k2 in range(mf // 4):
                            ks = slice(2 * k2, 2 * k2 + 2)
                            ffsrc = (ffq_pre if (blk == 0 and k2 < 2)
                                     else ffq)
                            nc.tensor.matmul(zo, w2a_t[:, ks, :],
                                             ffsrc[:, ks, hs],
                                             start=(k2 == 0), stop=False,
                                             perf_mode=DR)
                        for k2 in range(mf // 2):
                            nc.tensor.matmul(zo, w2b_t[:, k2, :],
                                             ffb[:, k2, hs],
                                             start=False,
                                             stop=(k2 == mf // 2 - 1))
                        yt = ypool.tile([P, NS], F32)
                        # y = zo/(s2*sp) + (x+h)  (bf16 residual; both W2
                        # halves carry the s2*sp scale)
                        nc.vector.scalar_tensor_tensor(
                            yt, zo, 1.0 / (s2 * sp), xnew_bf[:, m, gs],
                            op0=MULT, op1=ADD)
                        nc.sync.dma_start(y[:, m, gs], yt)

    nc.finalize()
    return nc


def _pack_lhsT(w, kd, dtype):
    # [K, M] -> [128, K/128, M] with [p, k, m] = w[k*128+p, m]
    K, M = w.shape
    return np.ascontiguousarray(
        w.reshape(kd, P, M).transpose(1, 0, 2)).astype(dtype)


def _quant(w):
    # power-of-2 scale mapping absmax into (90, 180] (fp8e4 max is 240)
    am = float(np.abs(w).max())
    s = 2.0 ** int(np.floor(np.log2(180.0 / am)))
    return (w * s).astype(np.float32), s


W_WARM = 32
SP = 8.0  # extra scale on the fp8-quantized silu(z1)*z3 product


def _prep_core_inputs(x, Wg, bg, Wc, bc, n1_w, n2_w, W1, W3, W2):
    B, L, D = x.shape
    DFF = W1.shape[1]
    kd, mf = D // P, DFF // P
    T_my = L // 2

    w1_s, s1 = _quant(n2_w[:, None] * W1)
    w3_s, s3 = _quant(n2_w[:, None] * W3)
    w2a_s, s2 = _quant(W2[:DFF // 2])
    scales = (s1, s3, s2, SP)

    wg_h = _pack_lhsT(n1_w[:, None] * Wg, kd, ml_dtypes.bfloat16)
    wc_h = _pack_lhsT(n1_w[:, None] * Wc, kd, ml_dtypes.bfloat16)
    w1_h = _pack_lhsT(w1_s, kd, E4NP)
    w3_h = _pack_lhsT(w3_s, kd, E4NP)
    w2a_h = _pack_lhsT(w2a_s, mf // 2, E4NP)
    w2b_h = _pack_lhsT(W2[DFF // 2:] * (s2 * SP), mf // 2,
                       ml_dtypes.bfloat16)
    bias_h = np.ascontiguousarray(np.stack(
        [bg.reshape(kd, P).T, bc.reshape(kd, P).T],
        axis=1)).astype(np.float32)

    assert np.all(bc == 0.0), "zero-warmup trick requires bc == 0"

    in_maps = []
    for c in range(8):
        b, s = c // 2, c % 2
        if s == 1:
            xb = x[b][T_my - W_WARM:]
        else:
            xb = np.concatenate(
                [np.zeros((W_WARM, D), np.float32), x[b][:T_my]], axis=0)
        xt_h = np.ascontiguousarray(
            xb.T.reshape(kd, P, T_my + W_WARM).transpose(1, 0, 2)
        ).astype(ml_dtypes.bfloat16)
        in_maps.append({"xt": xt_h, "wg": wg_h, "wc": wc_h, "bias": bias_h,
                       "w1": w1_h, "w3": w3_h, "w2a": w2a_h, "w2b": w2b_h})
    return in_maps, scales


_NC_CACHE = {}


def kernel(x, Wg, bg, Wc, bc, n1_w, n2_w, W1, W3, W2, _collect_perf=None):
    from concourse.bass_utils import run_bass_kernel_spmd

    x = np.asarray(x, np.float32)
    B, L, D = x.shape
    DFF = np.asarray(W1).shape[1]
    T_my = L // 2

    in_maps, scales = _prep_core_inputs(
        x, *[np.asarray(a, np.float32) for a in
             (Wg, bg, Wc, bc, n1_w, n2_w, W1, W3, W2)])

    key = (D, DFF, L, scales)
    if key not in _NC_CACHE:
        _NC_CACHE[key] = build_nc(D, DFF, T_my, W_WARM, *scales)
    nc = _NC_CACHE[key]

    res = run_bass_kernel_spmd(nc, in_maps, core_ids=list(range(8)))
    if _collect_perf is not None:
        _collect_perf.append(res)

    out = np.empty((B, L, D), np.float32)
    for c in range(8):
        b, s = c // 2, c % 2
        yc = res.results[c]["y"]  # [P, kd, T_my]
        out[b, s * T_my:(s + 1) * T_my] = (
            yc.transpose(2, 1, 0).reshape(T_my, D))
    return out
